# revision 22
# baseline (speedup 1.0000x reference)
"""COO SpMM (gnn message passing aggregator) on 8 trn2 NeuronCores.

out = A @ x where A is sparse COO (rows sorted): out[r] += vals[e] * x[cols[e]].

Measured: ~292 us HW exec (8-core SPMD, min of 5), rel err 2.4e-3 (bf16)
vs the fp32 reference; the v1 baseline (fp32 gathers + on-chip DVE/ACT
S-builds) measured ~540-625 us.

Design notes (evidence from perfetto traces of each iteration):
- The serial resource is GPSIMD: dma_gather descriptor emission runs on
  one Q7 core pair per SWDGE queue, and the engine dispatches only ~2
  DMAGatherAnt deep, so consecutive calls overlap pairwise at best.
  Near-equal-sized calls (each group's L stream split La/Lb, H whole;
  sizes ~6:5:6 chunks) on rotating queues get ~2x emission overlap.
- S scatter matrices are host-computed data (f(vals, rows) only) and
  stream from HBM as bf16 tiles per block. Building them on-chip (DVE
  is_equal) stalls the SWDGE ring writes through the shared POOL SBUF
  port and measured 1.7x worse; streaming them leaves DVE fully idle.
  The DMA engines run ~70-100% busy carrying gathers + S.
- x is bf16 in HBM: dma_gather moves 256B/edge; matmuls are bf16 into
  fp32 PSUM. fp8 fails the 2e-2 tolerance (measured 4e-2 numerically).
- Rows are reassigned to 128-row blocks host-side (greedy vector bin
  packing of per-row low/high edge counts, independent Poissons) so the
  uniform per-block chunk counts KL=11/KH=6 carry ~6% padding instead
  of ~19%. Output rows are un-permuted on host. Edges with equal
  (block, col) share one gather slot; equal (block, col, row) merge.
- GROUPS ramps 1,1,2,3,5,...,5,2,2,1,1,1: tiny first groups cut the
  startup bubble (the SWDGE doorbell fires at end-of-call, so block 0
  waits for its whole call to emit AND drain), tapered last groups cut
  the drain+matmul tail.
- Mid-call padding uses index 0 (gathers x[0], killed by S=0); trailing
  padding is -1 (trimmed by ucode). A call whose slot region grows past
  every earlier use of its rotated tile slot emits its full region so
  no uninitialized SBUF (potential NaN bits) ever reaches the PE.
- Per block: KL+KH matmuls accumulate S_chunk.T @ gathered_chunk into a
  PSUM tile [128 rows x 128 feat]; ACT drains, HWDGE stores; host
  concatenates per-core outputs, un-permutes rows, trims to 50000.
"""

import os
import numpy as np
import ml_dtypes
from contextlib import ExitStack

import concourse.tile as tile
from concourse import bacc, mybir
from concourse.bass_utils import run_bass_kernel_spmd

N_NODES = 50000
N_EDGES = 800000
D = 128
NCORES = 8
BLK = 128
NBLK = 49                 # blocks per core
RPC = NBLK * BLK          # 6272 rows per core
HI = 32768                # int16 index range split
# blocks per gather call-triple: tiny first groups fill the pipeline fast
# (the SWDGE doorbell fires at end-of-call, so block 0 waits for its whole
# call to emit AND drain); small last groups shrink the drain+matmul tail.
# Each group's L stream is split into two sub-calls (La/Lb) so consecutive
# calls are near-equal-sized: the GPSIMD sequencer runs ~2 DMAGatherAnt
# deep, so every other call's emission hides under its predecessor.
GROUPS = [1, 1, 2, 3] + [5] * 7 + [2, 2, 1, 1, 1]
assert sum(GROUPS) == NBLK
NGRP = len(GROUPS)
GSTART = [sum(GROUPS[:i]) for i in range(NGRP)]   # first block of group
NGB = 5                   # gather tile pool buffers per stream
PREF = 4                  # groups issued ahead of the block loop

last_exec_ns = None

bf16 = ml_dtypes.bfloat16


def _assign_blocks(li, hi, capL, capH):
    """Greedy balanced assignment of 6272 local rows to 49 blocks of 128.

    li/hi: per-local-row low/high edge counts. Returns slot_of[row] ->
    block*128 + pos. Balances both nL_b and nH_b under (capL, capH).
    """
    nrows = len(li)
    order = np.argsort(-(li + hi), kind="stable")
    nL = np.zeros(NBLK, np.int64)
    nH = np.zeros(NBLK, np.int64)
    cnt = np.zeros(NBLK, np.int64)
    slot_of = np.empty(nrows, np.int64)
    for r in order:
        # feasible blocks: cnt < 128; cost = max fractional load after adding
        mask = cnt < BLK
        costL = (nL + li[r]) / capL
        costH = (nH + hi[r]) / capH
        cost = np.maximum(costL, costH) + (~mask) * 1e9
        b = int(np.argmin(cost))
        slot_of[r] = b * BLK + cnt[b]
        cnt[b] += 1
        nL[b] += li[r]
        nH[b] += hi[r]
    return slot_of, nL, nH


def _shard(rows, cols, vals):
    """Pack edges into per-core structures; returns host data + layout."""
    core = rows // RPC
    local = rows - core * RPC

    # per-core row->block assignment balancing low/high edge counts
    low_all = cols < HI
    perm_slot = np.empty(NCORES * RPC, np.int64)   # local row -> block slot
    for c in range(NCORES):
        li = np.bincount(local[(core == c) & low_all], minlength=RPC)
        hi_ = np.bincount(local[(core == c) & ~low_all], minlength=RPC)
        # capacities chosen for KL=11, KH=6; assignment adapts if overflow
        slot_of, _, _ = _assign_blocks(li, hi_, 11 * BLK, 6 * BLK)
        perm_slot[c * RPC:(c + 1) * RPC] = slot_of

    slot = perm_slot[rows]            # block*128 + pos within block
    blk = slot // BLK
    lr = (slot % BLK).astype(np.int64)
    high = (~low_all).astype(np.int64)

    # dedup: edges with the same (core, blk, stream, col, lr) merge (vals
    # summed); edges sharing (core, blk, stream, col) share one gather slot
    # (the S column gets one entry per destination row).
    k1 = (((core * NBLK + blk) * 2 + high) * N_NODES + cols) * BLK + lr
    u1, inv1 = np.unique(k1, return_inverse=True)
    vals_u = np.bincount(inv1, weights=vals).astype(np.float32)
    lr_u = u1 % BLK
    t = u1 // BLK
    col_u = t % N_NODES
    t = t // N_NODES
    high_u = t % 2
    t = t // 2
    blk_u = t % NBLK
    core_u = t // NBLK
    low_u = high_u == 0

    # gather slots: rank of unique (core, blk, stream, col) within its group
    u2, inv2 = np.unique(u1 // BLK, return_inverse=True)
    g2 = u2 // N_NODES                # (core*NBLK + blk)*2 + stream
    counts = np.bincount(g2, minlength=NCORES * NBLK * 2)
    cl = counts[0::2]
    ch = counts[1::2]
    KL = max(1, int(np.ceil(cl.max() / BLK)))
    KH = max(1, int(np.ceil(ch.max() / BLK)))
    starts = np.zeros(NCORES * NBLK * 2, np.int64)
    np.cumsum(counts[:-1], out=starts[1:])
    j = (np.arange(len(u2)) - starts[g2])[inv2]   # slot rank per entry

    # stream slot position: block-padded layout, block b chunk k slot p at
    # position (b*K + k)*128 + p  ==  b*K*128 + j
    posL = blk_u * (KL * BLK) + j     # valid where low
    posH = blk_u * (KH * BLK) + j

    SL = NBLK * KL * BLK              # L stream length per core
    SH = NBLK * KH * BLK

    idxL = np.zeros((NCORES, SL), np.int16)   # 0 = valid pad (x[0], S=0)
    idxH = np.zeros((NCORES, SH), np.int16)
    idxL[core_u[low_u], posL[low_u]] = col_u[low_u].astype(np.int16)
    idxH[core_u[~low_u], posH[~low_u]] = (col_u[~low_u] - HI).astype(np.int16)

    # valid-edge occupancy masks to place trailing -1s per call
    occL = np.zeros((NCORES, SL), bool)
    occH = np.zeros((NCORES, SH), bool)
    occL[core_u[low_u], posL[low_u]] = True
    occH[core_u[~low_u], posH[~low_u]] = True

    # per-call emitted counts: trailing padding after the last valid edge in
    # the call is -1 (ucode trims); mid padding stays idx 0 (gathers x[0],
    # killed by S=0). Slot tiles are memset once at startup (idle DVE), so
    # every call can -1-trim and shrink its static num_idxs to the max
    # count across cores (the idx preload in the ucode runs over the
    # STATIC size; only descgen honors the -1 trim / count register).
    def call_cnt(idx, occ, a, e):
        """Per-core emitted counts for call range [a, e); -1-trim tails."""
        out = np.zeros(NCORES, np.int32)
        for c in range(NCORES):
            oc = occ[c, a:e]
            last = np.nonzero(oc)[0]
            n = (int(last[-1]) + 1) if len(last) else 0
            idx[c, a + n:e] = -1
            out[c] = n
        return out

    # cnt layout: [La_g, Lb_g, H_g] per group (3*NGRP entries)
    cnts = np.zeros((NCORES, 3 * NGRP), np.int32)
    for g in range(NGRP):
        nb = GROUPS[g]
        ka = (nb * KL + 1) // 2          # La sub-call chunks
        aL = GSTART[g] * KL * BLK
        mL = aL + ka * BLK
        eL = aL + nb * KL * BLK
        aH = GSTART[g] * KH * BLK
        eH = aH + nb * KH * BLK
        cnts[:, 3 * g + 0] = call_cnt(idxL, occL, aL, mL)
        cnts[:, 3 * g + 1] = call_cnt(idxL, occL, mL, eL)
        cnts[:, 3 * g + 2] = call_cnt(idxH, occH, aH, eH)
    callmax = tuple(int(v) for v in cnts.max(axis=0))

    # S: per core [128, NBLK*(KL+KH)*128] bf16, matmul order: block-major,
    # L chunks then H chunks. A slot shared by several (col, lr) entries
    # gets one S entry per destination row.
    KT = KL + KH
    SW = NBLK * KT * BLK
    S = np.zeros((NCORES, BLK, SW), bf16)
    part = (j % BLK).astype(np.int64)
    kk = j // BLK
    colL = (blk_u * KT + kk) * BLK + lr_u
    colH = (blk_u * KT + KL + kk) * BLK + lr_u
    S[core_u[low_u], part[low_u], colL[low_u]] = vals_u[low_u].astype(bf16)
    S[core_u[~low_u], part[~low_u], colH[~low_u]] = vals_u[~low_u].astype(bf16)

    # wrap idx streams to [128, P/16] (idx i at [i%16, i//16], replicated x8)
    def wrap(a):
        P = a.shape[1]
        return np.tile(a.reshape(NCORES, P // 16, 16).transpose(0, 2, 1),
                       (1, 8, 1)).copy()

    return (wrap(idxL), wrap(idxH), S, cnts, callmax, KL, KH, perm_slot)


def _build(KL, KH, callmax):
    KT = KL + KH
    GMAX = max(GROUPS)
    PL = NBLK * KL * BLK
    PH = NBLK * KH * BLK
    SW = NBLK * KT * BLK
    f32 = mybir.dt.float32
    bf = mybir.dt.bfloat16

    nc = bacc.Bacc("TRN2", target_bir_lowering=False, debug=False,
                   num_devices=NCORES, dynamic_dma_scratch_size=65536,
                   num_swdge_queues=4, detect_race_conditions=False)
    x_ap = nc.dram_tensor("xb", [N_NODES, D], bf, kind="ExternalInput").ap()
    iL_ap = nc.dram_tensor("idxL", [128, PL // 16], mybir.dt.int16,
                           kind="ExternalInput").ap()
    iH_ap = nc.dram_tensor("idxH", [128, PH // 16], mybir.dt.int16,
                           kind="ExternalInput").ap()
    s_ap = nc.dram_tensor("s", [128, SW], bf, kind="ExternalInput").ap()
    cnt_ap = nc.dram_tensor("cnt", [1, 3 * NGRP], mybir.dt.int32,
                            kind="ExternalInput").ap()
    out_ap = nc.dram_tensor("out", [RPC, D], f32, kind="ExternalOutput").ap()
    out_v = out_ap.rearrange("(b p) d -> b p d", p=128)

    with tile.TileContext(nc) as tc:
        with ExitStack() as ctx:
            pp = ctx.enter_context(tc.tile_pool(name="persist", bufs=1))
            gpl = ctx.enter_context(tc.tile_pool(name="gatherL", bufs=1))
            gph = ctx.enter_context(tc.tile_pool(name="gatherH", bufs=1))
            sp = ctx.enter_context(tc.tile_pool(name="spool", bufs=4))
            ps = ctx.enter_context(tc.tile_pool(name="psum", bufs=8,
                                                space="PSUM"))
            stg = ctx.enter_context(tc.tile_pool(name="stage", bufs=4))

            cnt_t = pp.tile([1, 3 * NGRP], mybir.dt.int32)
            nc.sync.dma_start(cnt_t[:], cnt_ap[:])
            # idx tiles split per group: the first gather then waits only for
            # its own ~50KB slab instead of the whole 1.7MB idx stream, which
            # cut ~20us off the startup bubble.
            iLg = []
            iHg = []
            for g in range(NGRP):
                aL = GSTART[g] * KL * BLK // 16
                eL = (GSTART[g] + GROUPS[g]) * KL * BLK // 16
                t = pp.tile([128, eL - aL], mybir.dt.int16, name=f"iL{g}")
                nc.sync.dma_start(t[:], iL_ap[:, aL:eL])
                iLg.append(t)
                aH = GSTART[g] * KH * BLK // 16
                eH = (GSTART[g] + GROUPS[g]) * KH * BLK // 16
                t = pp.tile([128, eH - aH], mybir.dt.int16, name=f"iH{g}")
                nc.sync.dma_start(t[:], iH_ap[:, aH:eH])
                iHg.append(t)

            slotL = [gpl.tile([128, GMAX * KL, D], bf, name=f"pgl{i}",
                              tag=f"pgl{i}") for i in range(NGB)]
            slotH = [gph.tile([128, GMAX * KH, D], bf, name=f"pgh{i}",
                              tag=f"pgh{i}") for i in range(NGB)]
            # one-time zero of the slot tiles on the otherwise-idle DVE so
            # trimmed call tails never expose uninitialized SBUF to the PE
            # (S=0 kills finite garbage but not NaN bit patterns).
            for t in slotL + slotH:
                nc.vector.memset(t[:], 0)
            gtsL = [None] * NGRP
            gtsH = [None] * NGRP
            qn = [0]

            def gather(cnt_col, dst, src, idx_t, a, n):
                # static num_idxs shrunk to the max count across cores
                # (rounded to 16): the ucode's idx preload runs over the
                # static size, so this trims preload work the -1 sentinel
                # cannot. dst is sliced to the matching chunk count.
                ns = min(n, (callmax[cnt_col] + 15) // 16 * 16)
                if ns == 0:
                    return
                dst = dst[:, :(ns + BLK - 1) // BLK, :]
                # the inline reg_load between gathers is load-bearing: a
                # hoisted-registers variant (pure back-to-back DMAGatherAnt)
                # measured ~8% slower, likely decode/ENG_REG handoff.
                with nc.gpsimd.register(f"cr{cnt_col}") as r:
                    nc.gpsimd.reg_load(r, cnt_t[0:1, cnt_col:cnt_col + 1])
                    nc.gpsimd.dma_gather(
                        out_ap=dst,
                        in_ap=src,
                        idxs_ap=idx_t[:, a // 16:(a + ns) // 16],
                        num_idxs=ns,
                        num_idxs_reg=r,
                        elem_size=D,
                        single_packet=False,
                        queue_num=qn[0] % 4,
                    )
                qn[0] += 1

            def issue_calls(g):
                # La, H, Lb: near-equal sizes so each call's emission hides
                # under its predecessor on the 2-deep GPSIMD dispatch.
                nb = GROUPS[g]
                ka = (nb * KL + 1) // 2
                kb = nb * KL - ka
                gtsL[g] = slotL[g % NGB]
                gtsH[g] = slotH[g % NGB]
                gather(3 * g + 0, gtsL[g][:, :ka, :],
                       x_ap[:], iLg[g], 0, ka * BLK)
                gather(3 * g + 2, gtsH[g][:, :nb * KH, :],
                       x_ap[HI:, :], iHg[g], 0, nb * KH * BLK)
                gather(3 * g + 1, gtsL[g][:, ka:ka + kb, :],
                       x_ap[:], iLg[g], ka * BLK, kb * BLK)

            def do_block(b):
                g = max(i for i in range(NGRP) if GSTART[i] <= b)
                brel = b - GSTART[g]
                s_t = sp.tile([128, KT * BLK], bf, name=f"s{b}", tag="s")
                nc.sync.dma_start(s_t[:], s_ap[:, b * KT * BLK:(b + 1) * KT * BLK])
                pt = ps.tile([128, 128], f32)
                for k in range(KT):
                    if k < KL:
                        rhs = gtsL[g][:, brel * KL + k, :]
                    else:
                        rhs = gtsH[g][:, brel * KH + (k - KL), :]
                    nc.tensor.matmul(pt[:], lhsT=s_t[:, k * BLK:(k + 1) * BLK],
                                     rhs=rhs,
                                     start=(k == 0),
                                     stop=(k == KT - 1))
                ot = stg.tile([128, 128], f32)
                nc.scalar.copy(ot[:], pt[:])
                nc.sync.dma_start(out_v[b], ot[:])

            for g in range(min(PREF, NGRP)):
                issue_calls(g)
            for g in range(NGRP):
                if g + PREF < NGRP:
                    issue_calls(g + PREF)
                for brel in range(GROUPS[g]):
                    do_block(GSTART[g] + brel)

    nc.compile()
    return nc


_CACHE = {}


def kernel(x, vals, rows, cols):
    global last_exec_ns
    x = np.ascontiguousarray(np.asarray(x, dtype=np.float32))
    vals = np.asarray(vals, dtype=np.float32)
    rows = np.asarray(rows, dtype=np.int64)
    cols = np.asarray(cols, dtype=np.int64)
    assert x.shape == (N_NODES, D) and vals.shape == rows.shape == cols.shape \
        == (N_EDGES,)

    idxL, idxH, S, cnts, callmax, KL, KH, perm_slot = _shard(rows, cols, vals)
    xb = x.astype(bf16)

    key = (KL, KH, callmax)
    if key not in _CACHE:
        _CACHE[key] = _build(KL, KH, callmax)
    nc = _CACHE[key]

    in_maps = [
        {"xb": xb, "idxL": idxL[c], "idxH": idxH[c], "s": S[c],
         "cnt": cnts[c:c + 1]}
        for c in range(NCORES)
    ]

    trace = os.environ.get("KERNEL_PROFILE", "0") == "1"
    res = run_bass_kernel_spmd(nc, in_maps, core_ids=list(range(NCORES)),
                               trace=trace)
    last_exec_ns = res.exec_time_ns

    out = np.empty((NCORES * RPC, D), np.float32)
    for c in range(NCORES):
        blocked = res.results[c]["out"]          # rows in block-slot order
        out[c * RPC:(c + 1) * RPC] = blocked[perm_slot[c * RPC:(c + 1) * RPC]]
    return out[:N_NODES]



# revision 25
# speedup vs baseline: 1.2567x; 1.2567x over previous
"""COO SpMM (gnn message passing aggregator) on 8 trn2 NeuronCores.

out = A @ x where A is sparse COO (rows sorted): out[r] += vals[e] * x[cols[e]].

Measured: ~292 us HW exec (8-core SPMD, min of 5), rel err 2.4e-3 (bf16)
vs the fp32 reference; the v1 baseline (fp32 gathers + on-chip DVE/ACT
S-builds) measured ~540-625 us.

Design notes (evidence from perfetto traces of each iteration):
- The serial resource is GPSIMD: dma_gather descriptor emission runs on
  one Q7 core pair per SWDGE queue, and the engine dispatches only ~2
  DMAGatherAnt deep, so consecutive calls overlap pairwise at best.
  Near-equal-sized calls (each group's L stream split La/Lb, H whole;
  sizes ~6:5:6 chunks) on rotating queues get ~2x emission overlap.
- S scatter matrices are host-computed data (f(vals, rows) only) and
  stream from HBM as bf16 tiles per block. Building them on-chip (DVE
  is_equal) stalls the SWDGE ring writes through the shared POOL SBUF
  port and measured 1.7x worse; streaming them leaves DVE fully idle.
  The DMA engines run ~70-100% busy carrying gathers + S.
- x is bf16 in HBM: dma_gather moves 256B/edge; matmuls are bf16 into
  fp32 PSUM. fp8 fails the 2e-2 tolerance (measured 4e-2 numerically).
- Rows are reassigned to 128-row blocks host-side (greedy vector bin
  packing of per-row low/high edge counts, independent Poissons) so the
  uniform per-block chunk counts KL=11/KH=6 carry ~6% padding instead
  of ~19%. Output rows are un-permuted on host. Edges with equal
  (block, col) share one gather slot; equal (block, col, row) merge.
- GROUPS ramps 1,1,2,3,5,...,5,2,2,1,1,1: tiny first groups cut the
  startup bubble (the SWDGE doorbell fires at end-of-call, so block 0
  waits for its whole call to emit AND drain), tapered last groups cut
  the drain+matmul tail.
- Mid-call padding uses index 0 (gathers x[0], killed by S=0); trailing
  padding is -1 (trimmed by ucode). A call whose slot region grows past
  every earlier use of its rotated tile slot emits its full region so
  no uninitialized SBUF (potential NaN bits) ever reaches the PE.
  Replacing `grown` with startup DVE memsets + per-call static num_idxs
  shrunk to the cross-core max measured 368 us (+72): the memset WAW
  chains gate the early gathers and outweigh the preload savings.
- Coalescing the 33 per-group idx loads into 8 (3 slabs + bulk rest)
  plus S-tile prefetch bufs 4->5 measured 302 us (+7): the Sync
  sequencer convoy was not the actual startup gate.
- Per block: KL+KH matmuls accumulate S_chunk.T @ gathered_chunk into a
  PSUM tile [128 rows x 128 feat]; ACT drains, HWDGE stores; host
  concatenates per-core outputs, un-permutes rows, trims to 50000.
"""

import os
import numpy as np
import ml_dtypes
from contextlib import ExitStack

import concourse.tile as tile
from concourse import bacc, mybir
from concourse.bass_utils import run_bass_kernel_spmd

N_NODES = 50000
N_EDGES = 800000
D = 128
NCORES = 8
BLK = 128
NBLK = 49                 # blocks per core
RPC = NBLK * BLK          # 6272 rows per core
HI = 32768                # int16 index range split
# blocks per gather call-triple: tiny first groups fill the pipeline fast
# (the SWDGE doorbell fires at end-of-call, so block 0 waits for its whole
# call to emit AND drain); small last groups shrink the drain+matmul tail.
# Each group's L stream is split into two sub-calls (La/Lb) so consecutive
# calls are near-equal-sized: the GPSIMD sequencer runs ~2 DMAGatherAnt
# deep, so every other call's emission hides under its predecessor.
GROUPS = [1, 1, 2, 3] + [5] * 7 + [2, 2, 1, 1, 1]
assert sum(GROUPS) == NBLK
NGRP = len(GROUPS)
GSTART = [sum(GROUPS[:i]) for i in range(NGRP)]   # first block of group
NGB = 5                   # gather tile pool buffers per stream
PREF = 4                  # groups issued ahead of the block loop

last_exec_ns = None

bf16 = ml_dtypes.bfloat16


def _assign_blocks(li, hi, capL, capH):
    """Greedy balanced assignment of 6272 local rows to 49 blocks of 128.

    li/hi: per-local-row low/high edge counts. Returns slot_of[row] ->
    block*128 + pos. Balances both nL_b and nH_b under (capL, capH).
    """
    nrows = len(li)
    order = np.argsort(-(li + hi), kind="stable")
    nL = np.zeros(NBLK, np.int64)
    nH = np.zeros(NBLK, np.int64)
    cnt = np.zeros(NBLK, np.int64)
    slot_of = np.empty(nrows, np.int64)
    for r in order:
        # feasible blocks: cnt < 128; cost = max fractional load after adding
        mask = cnt < BLK
        costL = (nL + li[r]) / capL
        costH = (nH + hi[r]) / capH
        cost = np.maximum(costL, costH) + (~mask) * 1e9
        b = int(np.argmin(cost))
        slot_of[r] = b * BLK + cnt[b]
        cnt[b] += 1
        nL[b] += li[r]
        nH[b] += hi[r]
    return slot_of, nL, nH


def _shard(rows, cols, vals):
    """Pack edges into per-core structures; returns host data + layout."""
    core = rows // RPC
    local = rows - core * RPC

    # per-core row->block assignment balancing low/high edge counts
    low_all = cols < HI
    perm_slot = np.empty(NCORES * RPC, np.int64)   # local row -> block slot
    for c in range(NCORES):
        li = np.bincount(local[(core == c) & low_all], minlength=RPC)
        hi_ = np.bincount(local[(core == c) & ~low_all], minlength=RPC)
        # capacities chosen for KL=11, KH=6; assignment adapts if overflow
        slot_of, _, _ = _assign_blocks(li, hi_, 11 * BLK, 6 * BLK)
        perm_slot[c * RPC:(c + 1) * RPC] = slot_of

    slot = perm_slot[rows]            # block*128 + pos within block
    blk = slot // BLK
    lr = (slot % BLK).astype(np.int64)
    high = (~low_all).astype(np.int64)

    # dedup: edges with the same (core, blk, stream, col, lr) merge (vals
    # summed); edges sharing (core, blk, stream, col) share one gather slot
    # (the S column gets one entry per destination row).
    k1 = (((core * NBLK + blk) * 2 + high) * N_NODES + cols) * BLK + lr
    u1, inv1 = np.unique(k1, return_inverse=True)
    vals_u = np.bincount(inv1, weights=vals).astype(np.float32)
    lr_u = u1 % BLK
    t = u1 // BLK
    col_u = t % N_NODES
    t = t // N_NODES
    high_u = t % 2
    t = t // 2
    blk_u = t % NBLK
    core_u = t // NBLK
    low_u = high_u == 0

    # gather slots: rank of unique (core, blk, stream, col) within its group
    u2, inv2 = np.unique(u1 // BLK, return_inverse=True)
    g2 = u2 // N_NODES                # (core*NBLK + blk)*2 + stream
    counts = np.bincount(g2, minlength=NCORES * NBLK * 2)
    cl = counts[0::2]
    ch = counts[1::2]
    KL = max(1, int(np.ceil(cl.max() / BLK)))
    KH = max(1, int(np.ceil(ch.max() / BLK)))
    starts = np.zeros(NCORES * NBLK * 2, np.int64)
    np.cumsum(counts[:-1], out=starts[1:])
    j = (np.arange(len(u2)) - starts[g2])[inv2]   # slot rank per entry

    # stream slot position: block-padded layout, block b chunk k slot p at
    # position (b*K + k)*128 + p  ==  b*K*128 + j
    posL = blk_u * (KL * BLK) + j     # valid where low
    posH = blk_u * (KH * BLK) + j

    SL = NBLK * KL * BLK              # L stream length per core
    SH = NBLK * KH * BLK

    idxL = np.zeros((NCORES, SL), np.int16)   # 0 = valid pad (x[0], S=0)
    idxH = np.zeros((NCORES, SH), np.int16)
    idxL[core_u[low_u], posL[low_u]] = col_u[low_u].astype(np.int16)
    idxH[core_u[~low_u], posH[~low_u]] = (col_u[~low_u] - HI).astype(np.int16)

    # valid-edge occupancy masks to place trailing -1s per call
    occL = np.zeros((NCORES, SL), bool)
    occH = np.zeros((NCORES, SH), bool)
    occL[core_u[low_u], posL[low_u]] = True
    occH[core_u[~low_u], posH[~low_u]] = True

    # per-call emitted counts: trailing padding after the last valid edge in
    # the call is -1 (ucode trims); mid padding stays idx 0 (gathers x[0],
    # killed by S=0). A call whose slot region grows past every earlier use
    # of its (rotated) tile slot would expose uninitialized SBUF in its
    # trimmed tail, so such calls emit their full region (pad stays idx 0).
    grown = []
    slot_max = [0] * NGB
    for g in range(NGRP):
        s = g % NGB
        grown.append(GROUPS[g] > slot_max[s])
        slot_max[s] = max(slot_max[s], GROUPS[g])

    def call_cnt(idx, occ, a, e, full):
        """Per-core emitted counts for call range [a, e); -1-trim the tail
        unless `full` (first write into a grown slot region)."""
        out = np.zeros(NCORES, np.int32)
        for c in range(NCORES):
            if full:
                out[c] = e - a
                continue
            oc = occ[c, a:e]
            last = np.nonzero(oc)[0]
            n = (int(last[-1]) + 1) if len(last) else 0
            idx[c, a + n:e] = -1
            out[c] = n
        return out

    # cnt layout: [La_g, Lb_g, H_g] per group (3*NGRP entries)
    cnts = np.zeros((NCORES, 3 * NGRP), np.int32)
    for g in range(NGRP):
        nb = GROUPS[g]
        ka = (nb * KL + 1) // 2          # La sub-call chunks
        aL = GSTART[g] * KL * BLK
        mL = aL + ka * BLK
        eL = aL + nb * KL * BLK
        aH = GSTART[g] * KH * BLK
        eH = aH + nb * KH * BLK
        cnts[:, 3 * g + 0] = call_cnt(idxL, occL, aL, mL, grown[g])
        cnts[:, 3 * g + 1] = call_cnt(idxL, occL, mL, eL, grown[g])
        cnts[:, 3 * g + 2] = call_cnt(idxH, occH, aH, eH, grown[g])
    callmax = tuple(int(v) for v in cnts.max(axis=0))

    # S: per core [128, NBLK*(KL+KH)*128] bf16, matmul order: block-major,
    # L chunks then H chunks. A slot shared by several (col, lr) entries
    # gets one S entry per destination row.
    KT = KL + KH
    SW = NBLK * KT * BLK
    S = np.zeros((NCORES, BLK, SW), bf16)
    part = (j % BLK).astype(np.int64)
    kk = j // BLK
    colL = (blk_u * KT + kk) * BLK + lr_u
    colH = (blk_u * KT + KL + kk) * BLK + lr_u
    S[core_u[low_u], part[low_u], colL[low_u]] = vals_u[low_u].astype(bf16)
    S[core_u[~low_u], part[~low_u], colH[~low_u]] = vals_u[~low_u].astype(bf16)

    # wrap idx streams to [128, P/16] (idx i at [i%16, i//16], replicated x8)
    def wrap(a):
        P = a.shape[1]
        return np.tile(a.reshape(NCORES, P // 16, 16).transpose(0, 2, 1),
                       (1, 8, 1)).copy()

    return (wrap(idxL), wrap(idxH), S, cnts, callmax, KL, KH, perm_slot)


def _build(KL, KH, callmax):
    KT = KL + KH
    GMAX = max(GROUPS)
    PL = NBLK * KL * BLK
    PH = NBLK * KH * BLK
    SW = NBLK * KT * BLK
    f32 = mybir.dt.float32
    bf = mybir.dt.bfloat16

    nc = bacc.Bacc("TRN2", target_bir_lowering=False, debug=False,
                   num_devices=NCORES, dynamic_dma_scratch_size=65536,
                   num_swdge_queues=4, detect_race_conditions=False)
    x_ap = nc.dram_tensor("xb", [N_NODES, D], bf, kind="ExternalInput").ap()
    iL_ap = nc.dram_tensor("idxL", [128, PL // 16], mybir.dt.int16,
                           kind="ExternalInput").ap()
    iH_ap = nc.dram_tensor("idxH", [128, PH // 16], mybir.dt.int16,
                           kind="ExternalInput").ap()
    s_ap = nc.dram_tensor("s", [128, SW], bf, kind="ExternalInput").ap()
    cnt_ap = nc.dram_tensor("cnt", [1, 3 * NGRP], mybir.dt.int32,
                            kind="ExternalInput").ap()
    out_ap = nc.dram_tensor("out", [RPC, D], f32, kind="ExternalOutput").ap()
    out_v = out_ap.rearrange("(b p) d -> b p d", p=128)

    with tile.TileContext(nc) as tc:
        with ExitStack() as ctx:
            pp = ctx.enter_context(tc.tile_pool(name="persist", bufs=1))
            gpl = ctx.enter_context(tc.tile_pool(name="gatherL", bufs=1))
            gph = ctx.enter_context(tc.tile_pool(name="gatherH", bufs=1))
            sp = ctx.enter_context(tc.tile_pool(name="spool", bufs=4))
            ps = ctx.enter_context(tc.tile_pool(name="psum", bufs=8,
                                                space="PSUM"))
            stg = ctx.enter_context(tc.tile_pool(name="stage", bufs=4))

            cnt_t = pp.tile([1, 3 * NGRP], mybir.dt.int32)
            nc.sync.dma_start(cnt_t[:], cnt_ap[:])
            # idx tiles split per group: the first gather then waits only for
            # its own ~50KB slab instead of the whole 1.7MB idx stream, which
            # cut ~20us off the startup bubble.
            iLg = []
            iHg = []
            for g in range(NGRP):
                aL = GSTART[g] * KL * BLK // 16
                eL = (GSTART[g] + GROUPS[g]) * KL * BLK // 16
                t = pp.tile([128, eL - aL], mybir.dt.int16, name=f"iL{g}")
                nc.sync.dma_start(t[:], iL_ap[:, aL:eL])
                iLg.append(t)
                aH = GSTART[g] * KH * BLK // 16
                eH = (GSTART[g] + GROUPS[g]) * KH * BLK // 16
                t = pp.tile([128, eH - aH], mybir.dt.int16, name=f"iH{g}")
                nc.sync.dma_start(t[:], iH_ap[:, aH:eH])
                iHg.append(t)

            slotL = [gpl.tile([128, GMAX * KL, D], bf, name=f"pgl{i}",
                              tag=f"pgl{i}") for i in range(NGB)]
            slotH = [gph.tile([128, GMAX * KH, D], bf, name=f"pgh{i}",
                              tag=f"pgh{i}") for i in range(NGB)]
            gtsL = [None] * NGRP
            gtsH = [None] * NGRP
            qn = [0]

            def gather(cnt_col, dst, src, idx_t, a, n):
                # static num_idxs shrunk to the cross-core max count: the
                # ucode's idx preload runs over the STATIC size, so this
                # trims preload work the -1 sentinel cannot. Grown calls
                # have callmax == region size and are unchanged, so their
                # full emissions still initialize every slot region.
                ns = max(16, min(n, (callmax[cnt_col] + 15) // 16 * 16))
                dst = dst[:, :(ns + BLK - 1) // BLK, :]
                # the inline reg_load between gathers is load-bearing: a
                # hoisted-registers variant (pure back-to-back DMAGatherAnt)
                # measured ~8% slower, likely decode/ENG_REG handoff.
                with nc.gpsimd.register(f"cr{cnt_col}") as r:
                    nc.gpsimd.reg_load(r, cnt_t[0:1, cnt_col:cnt_col + 1])
                    nc.gpsimd.dma_gather(
                        out_ap=dst,
                        in_ap=src,
                        idxs_ap=idx_t[:, a // 16:(a + ns) // 16],
                        num_idxs=ns,
                        num_idxs_reg=r,
                        elem_size=D,
                        single_packet=False,
                        queue_num=qn[0] % 4,
                    )
                qn[0] += 1

            def issue_calls(g):
                # La, H, Lb: near-equal sizes so each call's emission hides
                # under its predecessor on the 2-deep GPSIMD dispatch.
                nb = GROUPS[g]
                ka = (nb * KL + 1) // 2
                kb = nb * KL - ka
                gtsL[g] = slotL[g % NGB]
                gtsH[g] = slotH[g % NGB]
                gather(3 * g + 0, gtsL[g][:, :ka, :],
                       x_ap[:], iLg[g], 0, ka * BLK)
                gather(3 * g + 2, gtsH[g][:, :nb * KH, :],
                       x_ap[HI:, :], iHg[g], 0, nb * KH * BLK)
                gather(3 * g + 1, gtsL[g][:, ka:ka + kb, :],
                       x_ap[:], iLg[g], ka * BLK, kb * BLK)

            def do_block(b):
                g = max(i for i in range(NGRP) if GSTART[i] <= b)
                brel = b - GSTART[g]
                s_t = sp.tile([128, KT * BLK], bf, name=f"s{b}", tag="s")
                nc.sync.dma_start(s_t[:], s_ap[:, b * KT * BLK:(b + 1) * KT * BLK])
                pt = ps.tile([128, 128], f32)
                for k in range(KT):
                    if k < KL:
                        rhs = gtsL[g][:, brel * KL + k, :]
                    else:
                        rhs = gtsH[g][:, brel * KH + (k - KL), :]
                    nc.tensor.matmul(pt[:], lhsT=s_t[:, k * BLK:(k + 1) * BLK],
                                     rhs=rhs,
                                     start=(k == 0),
                                     stop=(k == KT - 1))
                ot = stg.tile([128, 128], f32)
                nc.scalar.copy(ot[:], pt[:])
                nc.sync.dma_start(out_v[b], ot[:])

            for g in range(min(PREF, NGRP)):
                issue_calls(g)
            for g in range(NGRP):
                if g + PREF < NGRP:
                    issue_calls(g + PREF)
                for brel in range(GROUPS[g]):
                    do_block(GSTART[g] + brel)

    nc.compile()
    return nc


_CACHE = {}


def kernel(x, vals, rows, cols):
    global last_exec_ns
    x = np.ascontiguousarray(np.asarray(x, dtype=np.float32))
    vals = np.asarray(vals, dtype=np.float32)
    rows = np.asarray(rows, dtype=np.int64)
    cols = np.asarray(cols, dtype=np.int64)
    assert x.shape == (N_NODES, D) and vals.shape == rows.shape == cols.shape \
        == (N_EDGES,)

    idxL, idxH, S, cnts, callmax, KL, KH, perm_slot = _shard(rows, cols, vals)
    xb = x.astype(bf16)

    key = (KL, KH, callmax)
    if key not in _CACHE:
        _CACHE[key] = _build(KL, KH, callmax)
    nc = _CACHE[key]

    in_maps = [
        {"xb": xb, "idxL": idxL[c], "idxH": idxH[c], "s": S[c],
         "cnt": cnts[c:c + 1]}
        for c in range(NCORES)
    ]

    trace = os.environ.get("KERNEL_PROFILE", "0") == "1"
    res = run_bass_kernel_spmd(nc, in_maps, core_ids=list(range(NCORES)),
                               trace=trace)
    last_exec_ns = res.exec_time_ns

    out = np.empty((NCORES * RPC, D), np.float32)
    for c in range(NCORES):
        blocked = res.results[c]["out"]          # rows in block-slot order
        out[c * RPC:(c + 1) * RPC] = blocked[perm_slot[c * RPC:(c + 1) * RPC]]
    return out[:N_NODES]

